# revision 17
# baseline (speedup 1.0000x reference)
"""CrossNetwork (DCN) forward on 8 TRN2 NeuronCores.

Reference computation (per cross layer i, x0 = input):
    s_i = xl . w_i            (per-row scalar)
    xl  = x0 * s_i + b_i + xl

Algebraic collapse: xl_i = alpha_i * x0 + c_i with per-row scalar alpha_i
and a row-constant vector c_i = sum_{j<i} b_j. Hence:
    u_i       = x0 . w_i                      (3 dots per row, all vs x0)
    alpha_0   = 1,  alpha_{i+1} = alpha_i * (1 + u_i) + (c_i . w_i)
    out       = alpha_3 * x0 + c_3
One read of x, one write of out -> memory roofline.

Sharding: pure data parallel over the batch dim, weights replicated.

The b == 0 specialization (the reference always passes b = 0) runs the
whole pipeline in bf16: x is cast fp32->bf16 inside the SDMA datapath on
load (SWDGE cast DMA — measured FASTER than a plain fp32 load because
DMA cost scales with HBM-side + SBUF-side bytes summed), the dots are
computed with fp32 accumulators on DVE / ACT, and the output is computed
and stored as bf16 (half the HBM write traffic), then upcast to fp32 on
the host during the unshard step.  At fp32 the kernel is DMA-bound at
~150 us per pass; the bf16 pipeline measures ~78 us (HW For_i-loop
slope), with DVE ~75 us and ACT ~77 us busy right under the ~80 us
cast-constrained DMA floor.  Accuracy: bf16 rounding is ~0.5% of each
element against the 2e-2 relative-error budget (measured 5.3e-3).

The general-b path keeps full fp32 math.
"""

import contextlib

import numpy as np

import concourse.bacc as bacc
import concourse.mybir as mybir
import concourse.tile as tile
from concourse.bass_utils import run_bass_kernel_spmd

N_CORES = 8
B, D, CROSS = 16384, 2048, 3
P = 128
F32 = mybir.dt.float32
BF16 = mybir.dt.bfloat16


G = 4  # tiles per DMA group (copy probes; the bf16 kernel defaults to 2)


def build_body_zero_b(tc, x_ap, w_ap, b_ap, out_ap, rows, reps=1, loop_reps=1,
                      passes_per_iter=1, stt_every=3, group=2):
    """b == 0, bf16 pipeline: out = alpha3 * x, alpha3 = (1+u0)(1+u1)(1+u2).

    x is cast fp32->bf16 inside the SDMA datapath (SWDGE cast DMA, issued
    from Pool), G=4 tiles per DMA so the Q7 descriptor-emission overhead
    amortizes.  The three dots per tile are split across engines -- the
    DVE stt (mult+accum) op has no 2x uop so it runs at 1x even in bf16,
    while plain tensor_tensor gets 2x and tensor_scalar 4x; Pool compute
    is 5-10x below its cost-model rate on real HW so it only issues DMA:

        u0: DVE stt (1x, fused accumulate), except every stt_every-th
            tile, which goes the u1/u2 route to balance DVE vs ACT
        u1, u2: DVE tensor_tensor mult (2x bf16) + ACT accumulate-copy

    Recurrence: t1 = 1+u0 on DVE; a2 = t1*u1 + t1 alternating ACT
    (bias-AP fused Identity) / DVE (stt); a3 = a2*u2 + a2 on DVE.
    Finals: DVE tensor_scalar (4x bf16).  Stores: bf16 via sync HWDGE,
    G tiles per DMA.

    reps > 1 unrolls the main loop in-NEFF; loop_reps > 1 wraps
    passes_per_iter unrolled passes in a hardware For_i loop
    (benchmarking only).
    """
    nc = tc.nc
    GG = group
    nt = rows // P
    ng = nt // GG
    Al = mybir.AluOpType
    Act = mybir.ActivationFunctionType

    with contextlib.ExitStack() as ctx:
        const = ctx.enter_context(tc.tile_pool(name="const", bufs=1))
        xpool = ctx.enter_context(
            tc.tile_pool(name="x", bufs={1: 12, 2: 6, 4: 3}.get(GG, 2)))
        ypool = ctx.enter_context(
            tc.tile_pool(name="y", bufs={1: 8, 2: 4, 4: 2}.get(GG, 2)))
        s0pool = ctx.enter_context(tc.tile_pool(name="scr0", bufs=3))
        s1pool = ctx.enter_context(tc.tile_pool(name="scr1", bufs=3))
        sapool = ctx.enter_context(tc.tile_pool(name="scra", bufs=4))
        upool = ctx.enter_context(tc.tile_pool(name="u", bufs=24))

        # Replicate w_i across partitions with stride-0 DMA reads of the
        # DRAM row, cast fp32->bf16 in the DMA (SWDGE only).
        wbc = []
        for i in range(CROSS):
            wt = const.tile([P, D], BF16, tag=f"w{i}")
            nc.gpsimd.dma_start(out=wt[:], in_=w_ap[i : i + 1, :].to_broadcast([P, D]))
            wbc.append(wt)

        def one_pass():
            for g in range(ng):
                xg = xpool.tile([P, GG * D], BF16, tag="x")
                nc.gpsimd.dma_start(
                    out=xg[:].rearrange("p (c d) -> p c d", c=GG),
                    in_=x_ap[g * GG * P : (g + 1) * GG * P, :].rearrange(
                        "(c p) d -> p c d", p=P),
                )
                yg = ypool.tile([P, GG * D], BF16, tag="y")
                for c in range(GG):
                    t = g * GG + c
                    xt = xg[:, c * D : (c + 1) * D]
                    u = upool.tile([P, 4], F32, tag="u")

                    # Dots: Pool compute is ~5-10x below its cost-model rate
                    # for [P, D] bf16 ops on real HW, so everything runs on
                    # DVE / ACT.  u0: fused stt on DVE (1x) for 12 of 16
                    # tiles; the other dots as DVE tt mult (2x bf16) + ACT
                    # accumulate-copy, balancing DVE ~74us vs ACT ~73us.
                    def dot(i, fused):
                        if fused:
                            scr = s0pool.tile([P, D], BF16, tag="scr0")
                            nc.vector.scalar_tensor_tensor(
                                out=scr[:], in0=xt, scalar=0.0, in1=wbc[i][:],
                                op0=Al.bypass, op1=Al.mult,
                                accum_out=u[:, i : i + 1],
                            )
                        else:
                            scr = s1pool.tile([P, D], BF16, tag="scr1")
                            nc.vector.tensor_tensor(
                                out=scr[:], in0=xt, in1=wbc[i][:], op=Al.mult)
                            scra = sapool.tile([P, D], BF16, tag="scra")
                            nc.scalar.activation(scra[:], scr[:], Act.Copy,
                                                 accum_out=u[:, i : i + 1])

                    dot(0, fused=(stt_every == 0 or t % stt_every != 0))
                    dot(1, fused=False)
                    dot(2, fused=False)

                    # alpha3 = (1+u0)(1+u1)(1+u2): tiny [P,1] ops, split
                    # DVE / ACT (fused mult-add via stt / bias-AP Identity)
                    t1 = upool.tile([P, 1], F32, tag="t1")
                    nc.vector.tensor_scalar_add(t1[:], u[:, 0:1], 1.0)
                    a2 = upool.tile([P, 1], F32, tag="a2")
                    if t % 2 == 0:
                        nc.scalar.activation(a2[:], u[:, 1:2], Act.Identity,
                                             bias=t1[:], scale=t1[:])
                    else:
                        nc.vector.scalar_tensor_tensor(
                            out=a2[:], in0=u[:, 1:2], scalar=t1[:], in1=t1[:],
                            op0=Al.mult, op1=Al.add,
                        )
                    a3 = upool.tile([P, 1], F32, tag="a3")
                    nc.vector.scalar_tensor_tensor(
                        out=a3[:], in0=u[:, 2:3], scalar=a2[:], in1=a2[:],
                        op0=Al.mult, op1=Al.add,
                    )

                    # final: DVE tensor_scalar (4x bf16)
                    nc.vector.tensor_scalar_mul(
                        yg[:, c * D : (c + 1) * D], xt, a3[:])

                nc.sync.dma_start(
                    out=out_ap[g * GG * P : (g + 1) * GG * P, :].rearrange(
                        "(c p) d -> p c d", p=P),
                    in_=yg[:].rearrange("p (c d) -> p c d", c=GG),
                )

        if loop_reps > 1:
            with tc.For_i(0, loop_reps, 1) as _:
                for _ in range(passes_per_iter):
                    one_pass()
        else:
            for _ in range(reps):
                one_pass()


def build_body_copy(tc, x_ap, w_ap, b_ap, out_ap, rows, cast, store=True,
                    reps=1, loop_reps=1, passes_per_iter=1):
    """Pure DMA floor probe: load x (optionally cast fp32->bf16) and,
    unless store=False, store it back out.  store=False stores only one
    tile per pass (negligible write traffic) to isolate the load floor.
    Benchmarking only (output is just x)."""
    nc = tc.nc
    nt = rows // P
    ng = nt // G

    with contextlib.ExitStack() as ctx:
        xpool = ctx.enter_context(tc.tile_pool(name="x", bufs=4))
        dt = BF16 if cast else F32

        def one_pass():
            for g in range(ng):
                xg = xpool.tile([P, G * D], dt, tag="x")
                load_eng = nc.gpsimd if cast else nc.sync
                load_eng.dma_start(
                    out=xg[:].rearrange("p (c d) -> p c d", c=G),
                    in_=x_ap[g * G * P : (g + 1) * G * P, :].rearrange(
                        "(c p) d -> p c d", p=P),
                )
                store_eng = nc.sync if cast else nc.scalar
                if store:
                    store_eng.dma_start(
                        out=out_ap[g * G * P : (g + 1) * G * P, :].rearrange(
                            "(c p) d -> p c d", p=P),
                        in_=xg[:].rearrange("p (c d) -> p c d", c=G),
                    )
                else:
                    store_eng.dma_start(
                        out=out_ap[g * G * P : g * G * P + P, :],
                        in_=xg[:, 0:D],
                    )

        if loop_reps > 1:
            with tc.For_i(0, loop_reps, 1) as _:
                for _ in range(passes_per_iter):
                    one_pass()
        else:
            for _ in range(reps):
                one_pass()


def build_body_zero_b_f32(tc, x_ap, w_ap, b_ap, out_ap, rows, reps=1,
                          loop_reps=1, passes_per_iter=1):
    """b == 0 fp32 reference pipeline (previous best, kept for A/B)."""
    nc = tc.nc
    nt = rows // P
    Al = mybir.AluOpType
    Act = mybir.ActivationFunctionType

    with contextlib.ExitStack() as ctx:
        const = ctx.enter_context(tc.tile_pool(name="const", bufs=1))
        xpool = ctx.enter_context(tc.tile_pool(name="x", bufs=6))
        ypool = ctx.enter_context(tc.tile_pool(name="y", bufs=6))
        spool = ctx.enter_context(tc.tile_pool(name="scr", bufs=2))
        sppool = ctx.enter_context(tc.tile_pool(name="scrp", bufs=3))
        sapool = ctx.enter_context(tc.tile_pool(name="scra", bufs=2))
        upool = ctx.enter_context(tc.tile_pool(name="u", bufs=24))

        wbc = []
        for i, eng in [(2, nc.scalar), (0, nc.sync), (1, nc.gpsimd)]:
            wt = const.tile([P, D], F32, tag=f"w{i}")
            eng.dma_start(out=wt[:], in_=w_ap[i : i + 1, :].to_broadcast([P, D]))
            wbc.append((i, wt))
        wbc = [t for _, t in sorted(wbc)]

        load_eng = {2: nc.gpsimd, 6: nc.gpsimd, 10: nc.gpsimd, 14: nc.gpsimd}
        store_eng = {}
        for i in (1, 5, 9):
            store_eng[i] = nc.scalar
        for i in (0, 4, 8, 12, 14):
            store_eng[i] = nc.gpsimd

        def one_pass():
            for t in range(nt):
                xt = xpool.tile([P, D], F32, tag="x")
                load_eng.get(t % 16, nc.sync).dma_start(
                    out=xt[:], in_=x_ap[t * P : (t + 1) * P, :]
                )

                u0 = upool.tile([P, 1], F32, tag="u0")
                scr0 = spool.tile([P, D], F32, tag="scr")
                nc.vector.scalar_tensor_tensor(
                    out=scr0[:], in0=xt[:], scalar=0.0, in1=wbc[0][:],
                    op0=Al.bypass, op1=Al.mult, accum_out=u0[:],
                )
                u1 = upool.tile([P, 1], F32, tag="u1")
                scr1 = spool.tile([P, D], F32, tag="scr")
                nc.vector.scalar_tensor_tensor(
                    out=scr1[:], in0=xt[:], scalar=0.0, in1=wbc[1][:],
                    op0=Al.bypass, op1=Al.mult, accum_out=u1[:],
                )
                scrp = sppool.tile([P, D], F32, tag="scrp")
                nc.gpsimd.tensor_tensor(out=scrp[:], in0=xt[:], in1=wbc[2][:],
                                        op=Al.mult)
                u2 = upool.tile([P, 1], F32, tag="u2")
                scra = sapool.tile([P, D], F32, tag="scra")
                nc.scalar.activation(scra[:], scrp[:], Act.Copy, accum_out=u2[:])

                t1 = upool.tile([P, 1], F32, tag="t1")
                nc.scalar.add(t1[:], u0[:], 1.0)
                t2 = upool.tile([P, 1], F32, tag="t2")
                nc.scalar.add(t2[:], u1[:], 1.0)
                a2 = upool.tile([P, 1], F32, tag="a2")
                nc.scalar.activation(a2[:], t2[:], Act.Identity, bias=0.0, scale=t1[:])
                t3 = upool.tile([P, 1], F32, tag="t3")
                nc.scalar.add(t3[:], u2[:], 1.0)
                a3 = upool.tile([P, 1], F32, tag="a3")
                nc.scalar.activation(a3[:], t3[:], Act.Identity, bias=0.0, scale=a2[:])

                yt = ypool.tile([P, D], F32, tag="y")
                if t % 4 == 3:
                    nc.vector.tensor_scalar_mul(yt[:], xt[:], a3[:])
                else:
                    nc.scalar.activation(yt[:], xt[:], Act.Copy, scale=a3[:])
                store_eng.get(t % 16, nc.sync).dma_start(
                    out=out_ap[t * P : (t + 1) * P, :], in_=yt[:]
                )

        if loop_reps > 1:
            with tc.For_i(0, loop_reps, 1) as _:
                for _ in range(passes_per_iter):
                    one_pass()
        else:
            for _ in range(reps):
                one_pass()


def build_body_general(tc, x_ap, w_ap, b_ap, out_ap, rows):
    """General-b path: full constants, final = ACT scale + Pool bias-add."""
    nc = tc.nc
    nt = rows // P
    Al = mybir.AluOpType
    Act = mybir.ActivationFunctionType

    with contextlib.ExitStack() as ctx:
        const = ctx.enter_context(tc.tile_pool(name="const", bufs=1))
        xpool = ctx.enter_context(tc.tile_pool(name="x", bufs=4))
        ypool = ctx.enter_context(tc.tile_pool(name="y", bufs=4))
        spool = ctx.enter_context(tc.tile_pool(name="scr", bufs=3))
        upool = ctx.enter_context(tc.tile_pool(name="u", bufs=16))

        # Load each tiny w_i / b_i row to partition 0, then replicate across
        # all 128 partitions on-chip (gpsimd partition_broadcast). The custom
        # op requires its input AP to start at partition 0, hence one [1, D]
        # tile per row. All row tiles are transient (pre pool).
        with tc.tile_pool(name="pre", bufs=1) as pre:
            wrow = []
            brow = []
            for i in range(CROSS):
                wr = pre.tile([1, D], F32, tag=f"wr{i}")
                nc.sync.dma_start(out=wr[:], in_=w_ap[i : i + 1, :])
                wrow.append(wr)
                br = pre.tile([1, D], F32, tag=f"br{i}")
                nc.sync.dma_start(out=br[:], in_=b_ap[i : i + 1, :])
                brow.append(br)

            wbc = []
            for i in range(CROSS):
                wt = const.tile([P, D], F32, tag=f"w{i}")
                nc.gpsimd.partition_broadcast(wt[:], wrow[i][:])
                wbc.append(wt)

            # row constants on [1, D]: c2 = b0 + b1, c3 = c2 + b2
            c2row = pre.tile([1, D], F32, tag="c2r")
            nc.vector.tensor_add(c2row[:], brow[0][:], brow[1][:])
            c3row = pre.tile([1, D], F32, tag="c3r")
            nc.vector.tensor_add(c3row[:], c2row[:], brow[2][:])
            c3bc = const.tile([P, D], F32, tag="c3")
            nc.gpsimd.partition_broadcast(c3bc[:], c3row[:])

            # k1 = b0 . w1, k2 = c2 . w2 (scalars), then replicate to [P, 1]
            k1row = pre.tile([1, 1], F32, tag="k1r")
            scr_k1 = pre.tile([1, D], F32, tag="scrr")
            nc.vector.scalar_tensor_tensor(
                out=scr_k1[:], in0=brow[0][:], scalar=0.0, in1=wrow[1][:],
                op0=Al.bypass, op1=Al.mult, accum_out=k1row[:],
            )
            k2row = pre.tile([1, 1], F32, tag="k2r")
            scr_k2 = pre.tile([1, D], F32, tag="scrr2")
            nc.vector.scalar_tensor_tensor(
                out=scr_k2[:], in0=c2row[:], scalar=0.0, in1=wrow[2][:],
                op0=Al.bypass, op1=Al.mult, accum_out=k2row[:],
            )
            k1bc = const.tile([P, 1], F32, tag="k1")
            nc.gpsimd.partition_broadcast(k1bc[:], k1row[:])
            k2bc = const.tile([P, 1], F32, tag="k2")
            nc.gpsimd.partition_broadcast(k2bc[:], k2row[:])

        for t in range(nt):
            xt = xpool.tile([P, D], F32, tag="x")
            nc.sync.dma_start(out=xt[:], in_=x_ap[t * P : (t + 1) * P, :])

            us = []
            for i in range(CROSS):
                u = upool.tile([P, 1], F32, tag=f"u{i}")
                scr = spool.tile([P, D], F32, tag="scr")
                nc.vector.scalar_tensor_tensor(
                    out=scr[:], in0=xt[:], scalar=0.0, in1=wbc[i][:],
                    op0=Al.bypass, op1=Al.mult, accum_out=u[:],
                )
                us.append(u)

            # alpha recurrence on ACT: a3 = ((1+u0)(1+u1) + k1)(1+u2) + k2
            t1 = upool.tile([P, 1], F32, tag="t1")
            nc.scalar.add(t1[:], us[0][:], 1.0)
            t2 = upool.tile([P, 1], F32, tag="t2")
            nc.scalar.add(t2[:], us[1][:], 1.0)
            a2 = upool.tile([P, 1], F32, tag="a2")
            nc.scalar.activation(a2[:], t2[:], Act.Identity, bias=k1bc[:], scale=t1[:])
            t3 = upool.tile([P, 1], F32, tag="t3")
            nc.scalar.add(t3[:], us[2][:], 1.0)
            a3 = upool.tile([P, 1], F32, tag="a3")
            nc.scalar.activation(a3[:], t3[:], Act.Identity, bias=k2bc[:], scale=a2[:])

            # out = alpha3 * x0 + c3: scale on ACT, bias-add in place on Pool
            yt = ypool.tile([P, D], F32, tag="y")
            nc.scalar.activation(yt[:], xt[:], Act.Copy, scale=a3[:])
            nc.gpsimd.tensor_tensor(out=yt[:], in0=yt[:], in1=c3bc[:], op=Al.add)
            nc.sync.dma_start(out=out_ap[t * P : (t + 1) * P, :], in_=yt[:])


_CACHE = {}


def get_nc(rows, zero_b=False, reps=1, loop_reps=1, passes_per_iter=1,
           variant="bf16"):
    key = (rows, zero_b, reps, loop_reps, passes_per_iter, variant)
    if key not in _CACHE:
        nc = bacc.Bacc(
            "TRN2",
            target_bir_lowering=False,
            debug=False,
            enable_asserts=False,
            num_devices=N_CORES,
        )
        out_dt = (BF16 if (zero_b and (variant.startswith("bf16") or variant
                           in ("copybf16", "loadcast"))) else F32)
        x = nc.dram_tensor("x", [rows, D], F32, kind="ExternalInput").ap()
        w = nc.dram_tensor("W", [CROSS, D], F32, kind="ExternalInput").ap()
        b = nc.dram_tensor("b", [CROSS, D], F32, kind="ExternalInput").ap()
        out = nc.dram_tensor("out", [rows, D], out_dt, kind="ExternalOutput").ap()
        with tile.TileContext(nc) as tc:
            if not zero_b:
                build_body_general(tc, x, w, b, out, rows)
            elif variant.startswith("bf16"):
                # bf16[sN][gM]: N = stt_every -- every Nth tile's u0 dot
                # is a DVE tt+ACT accum instead of a fused stt (0 = always
                # stt); M = tiles per DMA group
                import re as _re
                m = _re.fullmatch(r"bf16(?:s(\d+))?(?:g(\d+))?", variant)
                stt_every = int(m.group(1)) if m.group(1) is not None else 3
                group = int(m.group(2)) if m.group(2) is not None else 2
                build_body_zero_b(tc, x, w, b, out, rows, reps=reps,
                                  loop_reps=loop_reps,
                                  passes_per_iter=passes_per_iter,
                                  stt_every=stt_every, group=group)
            elif variant == "f32":
                build_body_zero_b_f32(tc, x, w, b, out, rows, reps=reps,
                                      loop_reps=loop_reps,
                                      passes_per_iter=passes_per_iter)
            elif variant in ("copybf16", "copyf32", "loadcast", "loadf32"):
                build_body_copy(tc, x, w, b, out, rows,
                                cast=(variant in ("copybf16", "loadcast")),
                                store=variant.startswith("copy"), reps=reps,
                                loop_reps=loop_reps,
                                passes_per_iter=passes_per_iter)
            else:
                raise ValueError(variant)
        nc.compile()
        _CACHE[key] = nc
    return _CACHE[key]


def run(x, W, b, trace=False, force_general=False, variant="bf16"):
    x = np.ascontiguousarray(np.asarray(x, dtype=np.float32))
    W = np.ascontiguousarray(np.asarray(W, dtype=np.float32))
    b = np.ascontiguousarray(np.asarray(b, dtype=np.float32))
    rows = x.shape[0] // N_CORES
    zero_b = (not force_general) and not b.any()
    nc = get_nc(rows, zero_b, variant=variant)
    in_maps = [
        {"x": x[i * rows : (i + 1) * rows], "W": W, "b": b} for i in range(N_CORES)
    ]
    try:
        res = run_bass_kernel_spmd(
            nc, in_maps, core_ids=list(range(N_CORES)), trace=trace
        )
    except ModuleNotFoundError:
        # BASS_TRACE in the environment routes through an NTFF profile hook
        # that is absent in some containers; fall back to an untraced run.
        import os

        os.environ["BASS_NEVER_TRACE"] = "1"
        res = run_bass_kernel_spmd(
            nc, in_maps, core_ids=list(range(N_CORES)), trace=False
        )
    out = np.concatenate([r["out"] for r in res.results], axis=0)
    if out.dtype != np.float32:
        out = out.astype(np.float32)
    return out, res


def kernel(x, W, b):
    out, _ = run(x, W, b)
    return out
